# revision 1
# baseline (speedup 1.0000x reference)
"""Dense-CRF relaxed Potts loss on 8 TRN2 NeuronCores — triangle version.

v2: exploits W's symmetry to nearly halve the exp work (the v1 bottleneck).

Math: loss*N = sum_ij s_i W_ij (1-s_j).  Split the 72x72 grid of 128-row
slab pairs by cyclic offset d = (b-a) mod 72:
  d=0   : self block, direct formula only
  1..35 : process pair once; direct + mirrored contribution:
            direct_ij = s_i W_ij (1-s_j)     (ACT bias log s_i + accum_out)
            mirror_ij = (1-s_i) W_ij s_j = h_i * direct_ij * r_j
          with r_j = s_j/(1-s_j): DVE tensor_tensor T*R (bf16 2x mode), then
          PE contracts h^T (T*R) with a single PSUM accumulator [2,512]
          (h in 2 bf16 limbs; column-folded sums; final reduce on host)
  d=36  : antipodal pairs appear once for each of the two owning slabs ->
          direct formula only (both directions covered)
SPMD uniformity: core k owns slabs {k+8t}; its copy of the B/R data is
ROTATED by k slabs so the program's column offsets (8t+d) mod 72 are
core-independent.

The z matmul uses a K=36 bf16 3-limb decomposition (features, sq, and the
log column terms each split into bf16 limbs, cross products paired so that
sum_k a_k[i] b_k[j] = -0.5*d2_ij + log1p(-s_j) to ~2^-24) giving fp32-grade
d2 at the bf16 rate of 1 cycle/row — 4x faster than a native fp32 matmul.
"""

import numpy as np
import ml_dtypes

import concourse.bacc as bacc
import concourse.tile as tile
from concourse import mybir
import concourse.bass_utils as bass_utils

BF16 = ml_dtypes.bfloat16

SIGMA_XY = 15.0
SIGMA_RGB = 0.125
H = W = 96
N = H * W                   # 9216
N_CORES = 8
NSLAB = N // 128            # 72 slabs of 128 rows
T_SLABS = NSLAB // N_CORES  # 9 per core
D_MAX = 36                  # offsets 0..36
GROUP_CAPS = (16, 12)       # alternating PSUM group sizes (4 banks / 3 banks)

_cached = {}


def _slab_runs(t):
    """Column-slab runs (m0, length, d0) for local slab t (start m = 8t)."""
    m0 = 8 * t
    if m0 + D_MAX <= NSLAB - 1:
        return [(m0, D_MAX + 1, 0)]
    l1 = NSLAB - m0
    return [(m0, l1, 0), (0, D_MAX + 1 - l1, l1)]


def _groups():
    """Compile-time schedule: list of (t, mb0, nb, mir_lo, mir_hi, parity)
    where mir_lo/mir_hi are group-local block bounds of the mirror range and
    parity selects which of the two alternating PSUM slots the group uses."""
    out = []
    parity = 0
    for t in range(T_SLABS):
        for (m0, L, d0) in _slab_runs(t):
            blo = max(0, 1 - d0)           # run-local mirror block range
            bhi = min(L, D_MAX - d0)
            b0 = 0
            while b0 < L:
                nb = min(GROUP_CAPS[parity], L - b0)
                # keep the kernel's very last group mirror-free (d=36 block
                # alone) so the tail is ACT-only, not an ACT->DVE->PE chain
                if t == T_SLABS - 1 and b0 < bhi < b0 + nb:
                    nb = bhi - b0
                mlo = max(b0, blo) - b0
                mhi = min(b0 + nb, bhi) - b0
                out.append((t, m0 + b0, nb, max(mlo, 0), max(mhi, 0), parity))
                b0 += nb
                parity ^= 1
    return out


def _build_module():
    groups = _groups()
    n_accd = len(groups)

    nc = bacc.Bacc(
        "TRN2",
        target_bir_lowering=False,
        debug=False,
        enable_asserts=False,
        num_devices=N_CORES,
    )
    f32 = mybir.dt.float32
    bf = mybir.dt.bfloat16
    a_src = nc.dram_tensor("a_src", [36, T_SLABS * 128], bf, kind="ExternalInput").ap()
    b_src = nc.dram_tensor("b_src", [36, N], bf, kind="ExternalInput").ap()
    logs_src = nc.dram_tensor("logs_src", [128, T_SLABS], f32, kind="ExternalInput").ap()
    r_src = nc.dram_tensor("r_src", [1, N], bf, kind="ExternalInput").ap()
    h_src = nc.dram_tensor("h_src", [128, 2 * T_SLABS], bf, kind="ExternalInput").ap()
    accd_out = nc.dram_tensor("accd_out", [128, n_accd], f32, kind="ExternalOutput").ap()
    m2_out = nc.dram_tensor("m2_out", [2, 512], f32, kind="ExternalOutput").ap()

    # count mirror matmul chunks to set start/stop flags
    n_mir = 0
    for (t, mb0, nb, mlo, mhi, parity) in groups:
        if mhi > mlo:
            w = (mhi - mlo) * 128
            n_mir += (w + 511) // 512

    with tile.TileContext(nc) as tc:
        with (
            tc.tile_pool(name="singles", bufs=1) as singles,
            tc.tile_pool(name="psA", bufs=1, space="PSUM") as psA_pool,
            tc.tile_pool(name="psB", bufs=1, space="PSUM") as psB_pool,
            tc.tile_pool(name="m2ps", bufs=1, space="PSUM") as m2_pool,
            tc.tile_pool(name="tpool", bufs=3) as t_pool,
        ):
            A = singles.tile([36, T_SLABS * 128], bf)
            B = singles.tile([36, N], bf)
            R = singles.tile([128, N], bf)
            LOGS = singles.tile([128, T_SLABS], f32)
            Hh = singles.tile([128, 2 * T_SLABS], bf)
            ACCD = singles.tile([128, n_accd], f32)
            M2 = m2_pool.tile([2, 512], f32)
            M2S = singles.tile([2, 512], f32)

            # trigger the ACT table load at t~0 via a dependency-free dummy
            DUM = singles.tile([128, 1], f32)
            nc.gpsimd.memset(DUM[:], 0.0)
            nc.scalar.activation(
                DUM[:], DUM[:], mybir.ActivationFunctionType.Exp, bias=0.0, scale=0.0
            )
            # few large DMAs (per-DMA descriptor cost dominates), but split B
            # so the first groups' columns land before the bulk transfer ends
            nc.sync.dma_start(B[:, 0:2048], b_src[:, 0:2048])
            nc.sync.dma_start(A[:], a_src)
            nc.sync.dma_start(LOGS[:], logs_src)
            nc.sync.dma_start(B[:, 2048:6144], b_src[:, 2048:6144])
            nc.sync.dma_start(Hh[:], h_src)
            nc.sync.dma_start(R[:, 0:4608], r_src[:, 0:4608].broadcast_to((128, 4608)))
            nc.sync.dma_start(B[:, 6144:N], b_src[:, 6144:N])
            nc.sync.dma_start(R[:, 4608:N], r_src[:, 4608:N].broadcast_to((128, 4608)))

            mm_i = 0
            for gi, (t, mb0, nb, mlo, mhi, parity) in enumerate(groups):
                lhsT = A[:, t * 128:(t + 1) * 128]
                width = nb * 128
                c0 = mb0 * 128
                pool_g = psA_pool if parity == 0 else psB_pool
                pt = pool_g.tile(
                    [128, GROUP_CAPS[parity] * 128], f32, tag=f"ps{parity}"
                )
                for q0 in range(0, width, 512):
                    qw = min(512, width - q0)
                    nc.tensor.matmul(
                        pt[:, q0:q0 + qw],
                        lhsT=lhsT,
                        rhs=B[:, c0 + q0:c0 + q0 + qw],
                        start=True,
                        stop=True,
                    )
                T = t_pool.tile([128, max(GROUP_CAPS) * 128], bf, tag="T")
                nc.scalar.activation(
                    T[:, 0:width],
                    pt[:, 0:width],
                    mybir.ActivationFunctionType.Exp,
                    bias=LOGS[:, t:t + 1],
                    scale=1.0,
                    accum_out=ACCD[:, gi:gi + 1],
                )
                if mhi > mlo:
                    o0 = mlo * 128
                    w = (mhi - mlo) * 128
                    TR = t_pool.tile([128, max(GROUP_CAPS) * 128], bf, tag="TR")
                    nc.vector.tensor_tensor(
                        TR[:, 0:w],
                        T[:, o0:o0 + w],
                        R[:, c0 + o0:c0 + o0 + w],
                        mybir.AluOpType.mult,
                    )
                    for q in range(0, w, 512):
                        qw = min(512, w - q)
                        nc.tensor.matmul(
                            M2[:, 0:qw],
                            lhsT=Hh[:, 2 * t:2 * t + 2],
                            rhs=TR[:, q:q + qw],
                            start=(mm_i == 0),
                            stop=(mm_i == n_mir - 1),
                            skip_group_check=True,
                        )
                        mm_i += 1

            assert mm_i == n_mir
            nc.vector.tensor_copy(M2S[:], M2[:])
            nc.sync.dma_start(accd_out, ACCD[:])
            nc.sync.dma_start(m2_out, M2S[:])

    nc.compile()
    return nc


def _limbs3(x):
    x = np.asarray(x, np.float64)
    l1 = x.astype(BF16)
    r = x - l1.astype(np.float64)
    l2 = r.astype(BF16)
    r -= l2.astype(np.float64)
    l3 = r.astype(BF16)
    return l1, l2, l3


def _limbs2(x):
    x = np.asarray(x, np.float64)
    l1 = x.astype(BF16)
    l2 = (x - l1.astype(np.float64)).astype(BF16)
    return l1, l2


def _prep_inputs(input, image):
    s = np.asarray(input, np.float32).reshape(N)
    img = np.asarray(image, np.float32).reshape(3, N)
    yy, xx = np.meshgrid(
        np.arange(H, dtype=np.float32), np.arange(W, dtype=np.float32), indexing="ij"
    )
    pos = np.stack([xx, yy], -1).reshape(N, 2) / np.float32(SIGMA_XY)
    feat = np.concatenate([pos, img.T / np.float32(SIGMA_RGB)], 1).astype(np.float32)
    sq = (feat * feat).sum(1, dtype=np.float32).astype(np.float32)

    fA, fB, fC = _limbs3(feat.T)
    sq1, sq2, sq3 = _limbs3(sq)
    lp = np.maximum(np.log1p(-s.astype(np.float64)), -500.0)
    t1, t2, t3 = _limbs3(-0.5 * sq.astype(np.float64) + lp)
    half = np.full(N, -0.5, BF16)
    one = np.ones(N, BF16)
    a = np.concatenate(
        [fA, fA, fB, fA, fC, fB, sq1[None], sq2[None], sq3[None],
         one[None], one[None], one[None]], axis=0).astype(BF16)
    b = np.concatenate(
        [fA, fB, fA, fC, fA, fB, half[None], half[None], half[None],
         t1[None], t2[None], t3[None]], axis=0).astype(BF16)
    s64 = s.astype(np.float64)
    with np.errstate(divide="ignore"):
        logs = np.maximum(np.log(s64), -500.0).astype(np.float32)
    r_full = np.minimum(s64 / np.maximum(1.0 - s64, 1e-300), 1e30).astype(BF16)
    h_full = np.minimum((1.0 - s64) / np.maximum(s64, 1e-300), 1e30)

    in_maps = []
    for k in range(N_CORES):
        own = [(k + 8 * t) % NSLAB for t in range(T_SLABS)]
        rot = [(k + m) % NSLAB for m in range(NSLAB)]
        rows = np.concatenate([np.arange(a0 * 128, (a0 + 1) * 128) for a0 in own])
        cols = np.concatenate([np.arange(m0 * 128, (m0 + 1) * 128) for m0 in rot])
        h1, h2 = _limbs2(h_full[rows])          # [1152] each
        h_packed = np.stack([h1.reshape(T_SLABS, 128), h2.reshape(T_SLABS, 128)], 1)
        # h_src[:, 2t] = limb1 of slab t, h_src[:, 2t+1] = limb2
        h_arr = np.ascontiguousarray(h_packed.reshape(T_SLABS * 2, 128).T.astype(BF16))
        in_maps.append(
            {
                "a_src": np.ascontiguousarray(a[:, rows]),
                "b_src": np.ascontiguousarray(b[:, cols]),
                "logs_src": np.ascontiguousarray(logs[rows].reshape(T_SLABS, 128).T),
                "r_src": np.ascontiguousarray(r_full[cols])[None, :],
                "h_src": h_arr,
            }
        )
    return in_maps


def _run(in_maps, **kwargs):
    if "nc" not in _cached:
        _cached["nc"] = _build_module()
    return bass_utils.run_bass_kernel_spmd(
        _cached["nc"], in_maps, core_ids=list(range(N_CORES)), **kwargs
    )


def kernel(input, image):
    assert input.shape == (1, 1, H, W) and image.shape == (1, 3, H, W)
    in_maps = _prep_inputs(input, image)
    res = _run(in_maps)
    total = 0.0
    for k in range(N_CORES):
        r = res.results[k]
        total += r["accd_out"].sum(dtype=np.float64)
        total += r["m2_out"].sum(dtype=np.float64)
    return np.array(total / N, dtype=np.float32)



# revision 24
# speedup vs baseline: 1.1366x; 1.1366x over previous
"""Dense-CRF relaxed Potts loss on 8 TRN2 NeuronCores — fp8 DoubleRow version.

v3: fp8e4m3 DoubleRow z-matmul (0.5 PE cycles/row), log s_i folded into the
matmul so activations need no per-slab bias and groups can span slabs
(fewer, larger ACT instructions), and direct row-sums load-balanced between
ACT accum_out and a PE ones-contraction into a second PSUM accumulator.

Math: loss*N = sum_ij s_i W_ij (1-s_j).  72 slabs of 128 rows; core k owns
slabs {k+8t}. Column data is rotated by k slabs so the SPMD program is
core-independent. Off-diagonal slab pairs are processed once (offsets
d=1..35 relative to the owning slab): the 'direct' term s_i W (1-s_j) comes
straight out of T = exp(z); the mirrored term (1-s_i) W s_j = h_i T_ij r_j
is computed as a DVE elementwise T*R followed by a PE contraction with the
2-limb h against a [2,512] PSUM accumulator (column-folded, partitions 0-1
of the 8th PSUM bank). Direct sums either ride the activation's accum_out
or a PE ones-contraction folded into partition 32 of the same bank. d=0 and
d=36 blocks are direct-only (d=36 appears once in each owning slab's run,
so both orientations are covered).

z = -0.5*d2 + log s_i + log1p(-s_j) is computed as a K=98 fp8e4m3
limb-pair decomposition (4 limbs/feature, cross pairs li+lj<=5, per-row
pow2 balancing so no limb under/overflows fp8; per-dim row ordering keeps
PE partial sums small for near pairs, where exp matters). fp32-grade z at
0.5 cycles/row via MatmulPerfMode.DoubleRow ([Kp=49, 2, .] layout).
"""

import numpy as np
import ml_dtypes

import concourse.bacc as bacc
import concourse.tile as tile
from concourse import mybir
import concourse.bass_utils as bass_utils

F8 = ml_dtypes.float8_e4m3fn
BF16 = ml_dtypes.bfloat16

SIGMA_XY = 15.0
SIGMA_RGB = 0.125
H = W = 96
N = H * W                   # 9216
N_CORES = 8
NSLAB = N // 128            # 72 slabs of 128 rows
T_SLABS = NSLAB // N_CORES  # 9 per core
D_MAX = 36
NLIMB = 4
KP = 49                     # fp8 row pairs: K=98
PSA = 16                    # psA pool blocks (4 banks)
PSB = 12                    # psB pool blocks (3 banks)
HEAD_SIZES = (4, 8)         # small first groups for a fast start

_cached = {}


def _stream():
    """Block stream: (t, m, mirror) with m the local column-slab index.
    Mirror blocks (d=1..35) slab-major first, then the 18 direct-only
    blocks (d=0 and d=36 of each slab) at the tail."""
    out = []
    for t in range(T_SLABS):
        for d in range(1, D_MAX):
            out.append((t, (8 * t + d) % NSLAB, True))
    for t in range(T_SLABS):
        out.append((t, 8 * t, False))
        out.append((t, (8 * t + D_MAX) % NSLAB, False))
    return out


def _plan():
    """Compile-time schedule.

    Returns list of group dicts:
      size, parity, zchunks [(o, nb, t, m0)], mpieces [(o, nb, t, m0)]
    where o/nb are block offsets/counts inside the group. zchunks respect
    PSUM bank alignment (cannot cross o%4 boundaries) and column/slab
    contiguity; mpieces are maximal same-slab column-contiguous mirror
    runs (SBUF-side, no bank constraint).
    """
    stream = _stream()
    groups = []
    i = 0
    parity = 0
    while i < len(stream):
        cap = (HEAD_SIZES[len(groups)] if len(groups) < len(HEAD_SIZES)
               else (PSA, PSB)[parity])
        nb = min(cap, len(stream) - i)
        blocks = stream[i:i + nb]

        def contiguous(x, y):
            return (x[0] == y[0] and x[2] == y[2]
                    and y[1] == x[1] + 1 and y[1] != 0)

        zchunks = []
        j = 0
        while j < nb:
            j2 = j + 1
            while (j2 < nb and j2 % 4 != 0
                   and contiguous(blocks[j2 - 1], blocks[j2])):
                j2 += 1
            zchunks.append((j, j2 - j, blocks[j][0], blocks[j][1]))
            j = j2
        mpieces = []
        j = 0
        while j < nb:
            if not blocks[j][2]:
                j += 1
                continue
            j2 = j + 1
            while (j2 < nb and blocks[j2][2]
                   and contiguous(blocks[j2 - 1], blocks[j2])):
                j2 += 1
            mpieces.append((j, j2 - j, blocks[j][0], blocks[j][1]))
            j = j2
        groups.append(dict(size=nb, parity=parity, zchunks=zchunks,
                           mpieces=mpieces))
        i += nb
        parity ^= 1
    return groups


def _routes(groups):
    """Direct-sum route per group: 'act' (accum_out) or 'pe' (ones
    contraction into M23 partition 32). PE takes mid-stream groups; ACT
    keeps the head (PE is cold) and the tail (short epilogue)."""
    n = len(groups)
    routes = []
    for gi in range(n):
        if gi < 2 or gi >= n - 3 or gi % 4 == 2:
            routes.append("act")
        else:
            routes.append("pe")
    return routes


def _build_module():
    groups = _plan()
    routes = _routes(groups)
    n_groups = len(groups)

    n_mir = sum(-(-p[1] * 128 // 512) for g in groups for p in g["mpieces"])
    n_ones = sum(-(-g["size"] * 128 // 512)
                 for gi, g in enumerate(groups) if routes[gi] == "pe")

    nc = bacc.Bacc(
        "TRN2",
        target_bir_lowering=False,
        debug=False,
        enable_asserts=False,
        num_devices=N_CORES,
    )
    f32 = mybir.dt.float32
    bf = mybir.dt.bfloat16
    f8 = mybir.dt.float8e4

    ab0_src = nc.dram_tensor("ab0_src", [KP, 2, 640], f8,
                             kind="ExternalInput").ap()
    a_src = nc.dram_tensor("a_src", [KP, 2, T_SLABS * 128], f8,
                           kind="ExternalInput").ap()
    b_src = nc.dram_tensor("b_src", [KP, 2, N], f8, kind="ExternalInput").ap()
    r_src = nc.dram_tensor("r_src", [1, N], bf, kind="ExternalInput").ap()
    h_src = nc.dram_tensor("h_src", [128, 2 * T_SLABS], bf,
                           kind="ExternalInput").ap()
    accd_out = nc.dram_tensor("accd_out", [128, n_groups], f32,
                              kind="ExternalOutput").ap()
    m23_out = nc.dram_tensor("m23_out", [2, 512], f32,
                             kind="ExternalOutput").ap()
    m3_out = nc.dram_tensor("m3_out", [1, 512], f32,
                            kind="ExternalOutput").ap()

    with tile.TileContext(nc) as tc:
        with (
            tc.tile_pool(name="singles", bufs=1) as singles,
            tc.tile_pool(name="psA", bufs=1, space="PSUM") as psA_pool,
            tc.tile_pool(name="psB", bufs=1, space="PSUM") as psB_pool,
            tc.tile_pool(name="m23ps", bufs=1, space="PSUM") as m23_pool,
            tc.tile_pool(name="tpool", bufs=6) as t_pool,
            tc.tile_pool(name="trpool", bufs=4) as tr_pool,
        ):
            AB0 = singles.tile([KP, 2, 640], f8)
            A = singles.tile([KP, 2, T_SLABS * 128], f8)
            B = singles.tile([KP, 2, N], f8)
            R = singles.tile([128, N], bf)
            Hh = singles.tile([128, 2 * T_SLABS], bf)
            ONES = singles.tile([128, 1], bf)
            ACCD = singles.tile([128, n_groups], f32)
            M23 = m23_pool.tile([33, 512], f32)
            M23S = singles.tile([2, 512], f32)
            M3S = singles.tile([1, 512], f32)

            # trigger the ACT exp table load immediately
            DUM = singles.tile([128, 1], f32)
            nc.gpsimd.memset(DUM[:], 0.0)
            nc.gpsimd.memset(ONES[:], 1.0)
            nc.gpsimd.memset(ACCD[:], 0.0)
            nc.scalar.activation(
                DUM[:], DUM[:], mybir.ActivationFunctionType.Exp,
                bias=0.0, scale=0.0,
            )
            # staged DMAs: B chunks stream on the SP queue in consumption
            # order; A/R/Hh go via the Pool (SWDGE) queue in parallel so
            # the head isn't serialized on one sequencer.
            nc.sync.dma_start(AB0[:], ab0_src)
            nc.sync.dma_start(B[:, :, 128:1664], b_src[:, :, 128:1664])
            nc.sync.dma_start(Hh[:], h_src)
            nc.sync.dma_start(B[:, :, 1664:3712], b_src[:, :, 1664:3712])
            nc.sync.dma_start(B[:, :, 3712:4608], b_src[:, :, 3712:4608])
            nc.sync.dma_start(B[:, :, 4608:6656], b_src[:, :, 4608:6656])
            nc.sync.dma_start(B[:, :, 6656:N], b_src[:, :, 6656:N])
            nc.sync.dma_start(B[:, :, 0:128], b_src[:, :, 0:128])
            nc.gpsimd.dma_start(A[:], a_src)
            nc.gpsimd.dma_start(R[:, 128:1664],
                                r_src[:, 128:1664].broadcast_to((128, 1536)))
            nc.gpsimd.dma_start(R[:, 1664:4608],
                                r_src[:, 1664:4608].broadcast_to((128, 2944)))
            nc.gpsimd.dma_start(R[:, 4608:6912],
                                r_src[:, 4608:6912].broadcast_to((128, 2304)))
            nc.gpsimd.dma_start(R[:, 6912:N],
                                r_src[:, 6912:N].broadcast_to((128, 2304)))
            nc.gpsimd.dma_start(R[:, 0:128],
                                r_src[:, 0:128].broadcast_to((128, 128)))

            mir_i = 0
            ones_i = 0
            pts = {}
            t_tiles = {}

            def emit_z(gi):
                g = groups[gi]
                parity = g["parity"]
                pool_g = psA_pool if parity == 0 else psB_pool
                cap = (PSA, PSB)[parity]
                pt = pool_g.tile([128, cap * 128], f32, tag=f"ps{parity}")
                pts[gi] = pt
                for (o, nb, t, m0) in g["zchunks"]:
                    q0, qw, c0 = o * 128, nb * 128, m0 * 128
                    if gi == 0:
                        lhsT, rhs = AB0[:, :, 0:128], AB0[:, :, c0:c0 + qw]
                    else:
                        lhsT = A[:, :, t * 128:(t + 1) * 128]
                        rhs = B[:, :, c0:c0 + qw]
                    nc.tensor.matmul(
                        pt[:, q0:q0 + qw],
                        lhsT=lhsT,
                        rhs=rhs,
                        start=True, stop=True,
                        perf_mode=mybir.MatmulPerfMode.DoubleRow,
                    )

            def emit_act(gi):
                g = groups[gi]
                width = g["size"] * 128
                T = t_pool.tile([128, PSA * 128], bf, tag="T")
                t_tiles[gi] = T
                accum = ACCD[:, gi:gi + 1] if routes[gi] == "act" else None
                nc.scalar.activation(
                    T[:, 0:width], pts.pop(gi)[:, 0:width],
                    mybir.ActivationFunctionType.Exp,
                    bias=0.0, scale=1.0,
                    accum_out=accum,
                )

            def emit_tail(gi):
                nonlocal mir_i, ones_i
                g = groups[gi]
                width = g["size"] * 128
                T = t_tiles.pop(gi)
                for (o, nb, t, m0) in g["mpieces"]:
                    o0, w, c0 = o * 128, nb * 128, m0 * 128
                    TR = tr_pool.tile([128, PSA * 128], bf, tag="TR")
                    nc.vector.tensor_tensor(
                        TR[:, o0:o0 + w], T[:, o0:o0 + w],
                        R[:, c0:c0 + w], mybir.AluOpType.mult,
                    )
                    for q in range(0, w, 512):
                        qw = min(512, w - q)
                        nc.tensor.matmul(
                            M23[0:2, 0:qw],
                            lhsT=Hh[:, 2 * t:2 * t + 2],
                            rhs=TR[:, o0 + q:o0 + q + qw],
                            start=(mir_i == 0),
                            stop=(mir_i == n_mir - 1),
                            skip_group_check=True,
                        )
                        mir_i += 1
                if routes[gi] == "pe":
                    for q in range(0, width, 512):
                        qw = min(512, width - q)
                        nc.tensor.matmul(
                            M23[32:33, 0:qw],
                            lhsT=ONES[:],
                            rhs=T[:, q:q + qw],
                            start=(ones_i == 0),
                            stop=(ones_i == n_ones - 1),
                            skip_group_check=True,
                            tile_position=(0, 32),
                        )
                        ones_i += 1

            # software-pipelined emission: z(g+2) goes to the PE queue
            # right after ACT(g) so PE never parks mirror work in front
            # of the next group's PSUM fill.
            emit_z(0)
            emit_z(1)
            for gi in range(n_groups):
                emit_act(gi)
                if gi + 2 < n_groups:
                    emit_z(gi + 2)
                emit_tail(gi)
                if gi == n_groups // 2:
                    nc.sync.dma_start(accd_out[:, 0:gi], ACCD[:, 0:gi])

            assert mir_i == n_mir and ones_i == n_ones
            half = n_groups // 2
            nc.vector.tensor_copy(M23S[:], M23[0:2, :])
            nc.vector.tensor_copy(M3S[:], M23[32:33, :])
            nc.sync.dma_start(m23_out, M23S[:])
            nc.sync.dma_start(m3_out, M3S[:])
            nc.sync.dma_start(accd_out[:, half:n_groups - 1],
                              ACCD[:, half:n_groups - 1])
            nc.scalar.dma_start(accd_out[:, n_groups - 1:n_groups],
                                ACCD[:, n_groups - 1:n_groups])

    nc.compile()
    return nc, routes, n_groups


def _limbs_f8(x, n=NLIMB):
    x = np.asarray(x, np.float64)
    out = []
    r = x
    for _ in range(n):
        l = r.astype(F8)
        out.append(l)
        r = r - l.astype(np.float64)
    return out


def _limbs2(x):
    x = np.asarray(x, np.float64)
    l1 = x.astype(BF16)
    l2 = (x - l1.astype(np.float64)).astype(BF16)
    return l1, l2


def _build_rows(feat, s):
    """fp8 limb-pair rows for z = -0.5*d2 + log s_i + log1p(-s_j).
    Per-dim interleave keeps PE partial sums small for near pairs."""
    sq_d = feat * feat
    s64 = np.asarray(s, np.float64)
    with np.errstate(divide="ignore"):
        logs = np.maximum(np.log(s64), -500.0)
        lp = np.maximum(np.log1p(-s64), -500.0)
    a_rows, b_rows = [], []
    pairs = [(i, j) for i in range(1, NLIMB + 1) for j in range(1, NLIMB + 1)
             if i + j <= NLIMB + 1]
    ones = np.ones(N, np.float64)
    two = (ones * 2.0).astype(F8)
    for d in range(5):
        for l in _limbs_f8(-0.5 * sq_d[:, d] * 0.5):
            a_rows.append(two)
            b_rows.append(l)
        fl = _limbs_f8(feat[:, d])
        for (li, lj) in pairs:
            p = 2 * (lj - li)
            a_rows.append((fl[li - 1].astype(np.float64) * 2.0**-p).astype(F8))
            b_rows.append((fl[lj - 1].astype(np.float64) * 2.0**p).astype(F8))
        for l in _limbs_f8(-0.5 * sq_d[:, d] * 0.5):
            a_rows.append(l)
            b_rows.append(two)
    for l in _limbs_f8(logs * 0.5):
        a_rows.append(l)
        b_rows.append(two)
    for l in _limbs_f8(lp * 0.5):
        a_rows.append(two)
        b_rows.append(l)
    a = np.stack(a_rows)
    b = np.stack(b_rows)
    assert a.shape[0] == 2 * KP, a.shape
    return a, b


def _prep_inputs(input, image):
    s = np.asarray(input, np.float32).reshape(N).astype(np.float64)
    img = np.asarray(image, np.float32).reshape(3, N).astype(np.float64)
    yy, xx = np.meshgrid(
        np.arange(H, dtype=np.float64), np.arange(W, dtype=np.float64),
        indexing="ij")
    pos = np.stack([xx, yy], -1).reshape(N, 2) / SIGMA_XY
    feat = np.concatenate([pos, img.T / SIGMA_RGB], 1)

    a_all, b_all = _build_rows(feat, s)     # [2*KP, N] fp8

    r_full = np.minimum(s / np.maximum(1.0 - s, 1e-300), 1e30).astype(BF16)
    h_full = np.minimum((1.0 - s) / np.maximum(s, 1e-300), 1e30)

    in_maps = []
    for k in range(N_CORES):
        own = [(k + 8 * t) % NSLAB for t in range(T_SLABS)]
        rot = [(k + m) % NSLAB for m in range(NSLAB)]
        rows = np.concatenate(
            [np.arange(a0 * 128, (a0 + 1) * 128) for a0 in own])
        cols = np.concatenate(
            [np.arange(m0 * 128, (m0 + 1) * 128) for m0 in rot])
        h1, h2 = _limbs2(h_full[rows])
        h_packed = np.stack(
            [h1.reshape(T_SLABS, 128), h2.reshape(T_SLABS, 128)], 1)
        h_arr = np.ascontiguousarray(
            h_packed.reshape(T_SLABS * 2, 128).T.astype(BF16))
        a_k = a_all[:, rows].reshape(KP, 2, T_SLABS * 128)
        b_k = b_all[:, cols].reshape(KP, 2, N)
        in_maps.append(
            {
                "ab0_src": np.ascontiguousarray(
                    np.concatenate([a_k[:, :, 0:128], b_k[:, :, 128:640]],
                                   axis=2)),
                "a_src": np.ascontiguousarray(
                    a_all[:, rows].reshape(KP, 2, T_SLABS * 128)),
                "b_src": np.ascontiguousarray(
                    b_all[:, cols].reshape(KP, 2, N)),
                "r_src": np.ascontiguousarray(r_full[cols])[None, :],
                "h_src": h_arr,
            }
        )
    return in_maps


def _get_module():
    if "nc" not in _cached:
        _cached["nc"], _cached["routes"], _cached["n_groups"] = \
            _build_module()
    return _cached["nc"], _cached["routes"], _cached["n_groups"]


def _run(in_maps, **kwargs):
    nc, _, _ = _get_module()
    return bass_utils.run_bass_kernel_spmd(
        nc, in_maps, core_ids=list(range(N_CORES)), **kwargs
    )


def kernel(input, image):
    assert input.shape == (1, 1, H, W) and image.shape == (1, 3, H, W)
    nc, routes, n_groups = _get_module()
    in_maps = _prep_inputs(input, image)
    res = _run(in_maps)
    act_cols = [gi for gi in range(n_groups) if routes[gi] == "act"]
    total = 0.0
    for k in range(N_CORES):
        r = res.results[k]
        total += r["accd_out"][:, act_cols].sum(dtype=np.float64)
        total += r["m23_out"].sum(dtype=np.float64)
        total += r["m3_out"].sum(dtype=np.float64)
    return np.array(total / N, dtype=np.float32)


# revision 29
# speedup vs baseline: 1.1784x; 1.0368x over previous
"""Dense-CRF relaxed Potts loss on 8 TRN2 NeuronCores — fp8 DoubleRow version.

v3: fp8e4m3 DoubleRow z-matmul (0.5 PE cycles/row), log s_i folded into the
matmul so activations need no per-slab bias and groups can span slabs
(fewer, larger ACT instructions), and direct row-sums load-balanced between
ACT accum_out and a PE ones-contraction into a second PSUM accumulator.

Math: loss*N = sum_ij s_i W_ij (1-s_j).  72 slabs of 128 rows; core k owns
slabs {k+8t}. Column data is rotated by k slabs so the SPMD program is
core-independent. Off-diagonal slab pairs are processed once (offsets
d=1..35 relative to the owning slab): the 'direct' term s_i W (1-s_j) comes
straight out of T = exp(z); the mirrored term (1-s_i) W s_j = h_i T_ij r_j
is computed as a DVE elementwise T*R followed by a PE contraction with the
2-limb h against a [2,512] PSUM accumulator (column-folded, partitions 0-1
of the 8th PSUM bank). Direct sums either ride the activation's accum_out
or a PE ones-contraction folded into partition 32 of the same bank. d=0 and
d=36 blocks are direct-only (d=36 appears once in each owning slab's run,
so both orientations are covered).

z = -0.5*d2 + log s_i + log1p(-s_j) is computed as a K=98 fp8e4m3
limb-pair decomposition (4 limbs/feature, cross pairs li+lj<=5, per-row
pow2 balancing so no limb under/overflows fp8; per-dim row ordering keeps
PE partial sums small for near pairs, where exp matters). fp32-grade z at
0.5 cycles/row via MatmulPerfMode.DoubleRow ([Kp=49, 2, .] layout).
"""

import numpy as np
import ml_dtypes

import concourse.bacc as bacc
import concourse.tile as tile
from concourse import mybir
import concourse.bass_utils as bass_utils

F8 = ml_dtypes.float8_e4m3fn
BF16 = ml_dtypes.bfloat16

SIGMA_XY = 15.0
SIGMA_RGB = 0.125
H = W = 96
N = H * W                   # 9216
N_CORES = 8
NSLAB = N // 128            # 72 slabs of 128 rows
T_SLABS = NSLAB // N_CORES  # 9 per core
D_MAX = 36
NLIMB = 4
KP = 49                     # fp8 row pairs: K=98
PSA = 16                    # psA pool blocks (4 banks)
PSB = 12                    # psB pool blocks (3 banks)
HEAD_SIZES = (4, 8)         # small first groups for a fast start

_cached = {}


def _stream():
    """Block stream: (t, m, mirror) with m the local column-slab index.
    Mirror blocks (d=1..35) slab-major first, then the 18 direct-only
    blocks (d=0 and d=36 of each slab) at the tail."""
    out = []
    for t in range(T_SLABS):
        for d in range(1, D_MAX):
            out.append((t, (8 * t + d) % NSLAB, True))
    for t in range(T_SLABS):
        out.append((t, 8 * t, False))
        out.append((t, (8 * t + D_MAX) % NSLAB, False))
    return out


def _plan():
    """Compile-time schedule.

    Returns list of group dicts:
      size, parity, zchunks [(o, nb, t, m0)], mpieces [(o, nb, t, m0)]
    where o/nb are block offsets/counts inside the group. zchunks respect
    PSUM bank alignment (cannot cross o%4 boundaries) and column/slab
    contiguity; mpieces are maximal same-slab column-contiguous mirror
    runs (SBUF-side, no bank constraint).
    """
    stream = _stream()
    groups = []
    i = 0
    parity = 0
    while i < len(stream):
        cap = (HEAD_SIZES[len(groups)] if len(groups) < len(HEAD_SIZES)
               else (PSA, PSB)[parity])
        nb = min(cap, len(stream) - i)
        blocks = stream[i:i + nb]

        def contiguous(x, y):
            return (x[0] == y[0] and x[2] == y[2]
                    and y[1] == x[1] + 1 and y[1] != 0)

        zchunks = []
        j = 0
        while j < nb:
            j2 = j + 1
            while (j2 < nb and j2 % 4 != 0
                   and contiguous(blocks[j2 - 1], blocks[j2])):
                j2 += 1
            zchunks.append((j, j2 - j, blocks[j][0], blocks[j][1]))
            j = j2
        mpieces = []
        j = 0
        while j < nb:
            if not blocks[j][2]:
                j += 1
                continue
            j2 = j + 1
            while (j2 < nb and blocks[j2][2]
                   and contiguous(blocks[j2 - 1], blocks[j2])):
                j2 += 1
            mpieces.append((j, j2 - j, blocks[j][0], blocks[j][1]))
            j = j2
        groups.append(dict(size=nb, parity=parity, zchunks=zchunks,
                           mpieces=mpieces))
        i += nb
        parity ^= 1
    return groups


def _routes(groups):
    """Direct-sum route per group: 'act' (accum_out) or 'pe' (ones
    contraction into M23 partition 32). PE takes mid-stream groups; ACT
    keeps the head (PE is cold) and the tail (short epilogue)."""
    n = len(groups)
    routes = []
    for gi in range(n):
        if gi < 2 or gi >= n - 3 or gi % 4 == 2:
            routes.append("act")
        else:
            routes.append("pe")
    return routes


def _build_module():
    groups = _plan()
    routes = _routes(groups)
    n_groups = len(groups)

    n_mir = sum(-(-p[1] * 128 // 512) for g in groups for p in g["mpieces"])
    n_ones = sum(-(-g["size"] * 128 // 512)
                 for gi, g in enumerate(groups) if routes[gi] == "pe")

    nc = bacc.Bacc(
        "TRN2",
        target_bir_lowering=False,
        debug=False,
        enable_asserts=False,
        num_devices=N_CORES,
    )
    f32 = mybir.dt.float32
    bf = mybir.dt.bfloat16
    f8 = mybir.dt.float8e4

    ab0_src = nc.dram_tensor("ab0_src", [KP, 2, 1664], f8,
                             kind="ExternalInput").ap()
    a_src = nc.dram_tensor("a_src", [KP, 2, T_SLABS * 128], f8,
                           kind="ExternalInput").ap()
    b_src = nc.dram_tensor("b_src", [KP, 2, N], f8, kind="ExternalInput").ap()
    r_src = nc.dram_tensor("r_src", [1, N], bf, kind="ExternalInput").ap()
    h_src = nc.dram_tensor("h_src", [128, 2 * T_SLABS], bf,
                           kind="ExternalInput").ap()
    accd_out = nc.dram_tensor("accd_out", [128, n_groups], f32,
                              kind="ExternalOutput").ap()
    m23_out = nc.dram_tensor("m23_out", [2, 512], f32,
                             kind="ExternalOutput").ap()
    m3_out = nc.dram_tensor("m3_out", [1, 512], f32,
                            kind="ExternalOutput").ap()

    with tile.TileContext(nc) as tc:
        with (
            tc.tile_pool(name="singles", bufs=1) as singles,
            tc.tile_pool(name="psA", bufs=1, space="PSUM") as psA_pool,
            tc.tile_pool(name="psB", bufs=1, space="PSUM") as psB_pool,
            tc.tile_pool(name="m23ps", bufs=1, space="PSUM") as m23_pool,
            tc.tile_pool(name="tpool", bufs=6) as t_pool,
            tc.tile_pool(name="trpool", bufs=4) as tr_pool,
        ):
            AB0 = singles.tile([KP, 2, 1664], f8)
            A = singles.tile([KP, 2, T_SLABS * 128], f8)
            B = singles.tile([KP, 2, N], f8)
            R = singles.tile([128, N], bf)
            Hh = singles.tile([128, 2 * T_SLABS], bf)
            ONES = singles.tile([128, 1], bf)
            ACCD = singles.tile([128, n_groups], f32)
            M23 = m23_pool.tile([33, 512], f32)
            M23S = singles.tile([2, 512], f32)
            M3S = singles.tile([1, 512], f32)

            # trigger the ACT exp table load immediately
            DUM = singles.tile([128, 1], f32)
            nc.gpsimd.memset(DUM[:], 0.0)
            nc.gpsimd.memset(ONES[:], 1.0)
            nc.gpsimd.memset(ACCD[:], 0.0)
            nc.scalar.activation(
                DUM[:], DUM[:], mybir.ActivationFunctionType.Exp,
                bias=0.0, scale=0.0,
            )
            # staged DMAs: B chunks stream on the SP queue in consumption
            # order; A/R/Hh go via the Pool (SWDGE) queue in parallel so
            # the head isn't serialized on one sequencer.
            nc.sync.dma_start(AB0[:], ab0_src)
            nc.sync.dma_start(B[:, :, 128:1664], b_src[:, :, 128:1664])
            nc.sync.dma_start(B[:, :, 1664:3712], b_src[:, :, 1664:3712])
            nc.sync.dma_start(Hh[:], h_src)
            nc.sync.dma_start(B[:, :, 3712:4608], b_src[:, :, 3712:4608])
            nc.sync.dma_start(B[:, :, 4608:6656], b_src[:, :, 4608:6656])
            nc.sync.dma_start(B[:, :, 6656:N], b_src[:, :, 6656:N])
            nc.sync.dma_start(B[:, :, 0:128], b_src[:, :, 0:128])
            nc.gpsimd.dma_start(A[:], a_src)
            nc.gpsimd.dma_start(R[:, 128:1664],
                                r_src[:, 128:1664].broadcast_to((128, 1536)))
            nc.gpsimd.dma_start(R[:, 1664:4608],
                                r_src[:, 1664:4608].broadcast_to((128, 2944)))
            nc.gpsimd.dma_start(R[:, 4608:6912],
                                r_src[:, 4608:6912].broadcast_to((128, 2304)))
            nc.gpsimd.dma_start(R[:, 6912:N],
                                r_src[:, 6912:N].broadcast_to((128, 2304)))
            nc.gpsimd.dma_start(R[:, 0:128],
                                r_src[:, 0:128].broadcast_to((128, 128)))

            mir_i = 0
            ones_i = 0
            pts = {}
            t_tiles = {}

            def emit_z(gi):
                g = groups[gi]
                parity = g["parity"]
                pool_g = psA_pool if parity == 0 else psB_pool
                cap = (PSA, PSB)[parity]
                pt = pool_g.tile([128, cap * 128], f32, tag=f"ps{parity}")
                pts[gi] = pt
                for (o, nb, t, m0) in g["zchunks"]:
                    q0, qw, c0 = o * 128, nb * 128, m0 * 128
                    if gi <= 1:
                        lhsT, rhs = AB0[:, :, 0:128], AB0[:, :, c0:c0 + qw]
                    else:
                        lhsT = A[:, :, t * 128:(t + 1) * 128]
                        rhs = B[:, :, c0:c0 + qw]
                    nc.tensor.matmul(
                        pt[:, q0:q0 + qw],
                        lhsT=lhsT,
                        rhs=rhs,
                        start=True, stop=True,
                        perf_mode=mybir.MatmulPerfMode.DoubleRow,
                    )

            def emit_act(gi):
                g = groups[gi]
                width = g["size"] * 128
                T = t_pool.tile([128, PSA * 128], bf, tag="T")
                t_tiles[gi] = T
                accum = ACCD[:, gi:gi + 1] if routes[gi] == "act" else None
                nc.scalar.activation(
                    T[:, 0:width], pts.pop(gi)[:, 0:width],
                    mybir.ActivationFunctionType.Exp,
                    bias=0.0, scale=1.0,
                    accum_out=accum,
                )

            def emit_tail(gi):
                nonlocal mir_i, ones_i
                g = groups[gi]
                width = g["size"] * 128
                T = t_tiles.pop(gi)
                for (o, nb, t, m0) in g["mpieces"]:
                    o0, w, c0 = o * 128, nb * 128, m0 * 128
                    TR = tr_pool.tile([128, PSA * 128], bf, tag="TR")
                    nc.vector.tensor_tensor(
                        TR[:, o0:o0 + w], T[:, o0:o0 + w],
                        R[:, c0:c0 + w], mybir.AluOpType.mult,
                    )
                    for q in range(0, w, 512):
                        qw = min(512, w - q)
                        nc.tensor.matmul(
                            M23[0:2, 0:qw],
                            lhsT=Hh[:, 2 * t:2 * t + 2],
                            rhs=TR[:, o0 + q:o0 + q + qw],
                            start=(mir_i == 0),
                            stop=(mir_i == n_mir - 1),
                            skip_group_check=True,
                        )
                        mir_i += 1
                if routes[gi] == "pe":
                    for q in range(0, width, 512):
                        qw = min(512, width - q)
                        nc.tensor.matmul(
                            M23[32:33, 0:qw],
                            lhsT=ONES[:],
                            rhs=T[:, q:q + qw],
                            start=(ones_i == 0),
                            stop=(ones_i == n_ones - 1),
                            skip_group_check=True,
                            tile_position=(0, 32),
                        )
                        ones_i += 1

            # software-pipelined emission: z(g+2) goes to the PE queue
            # right after ACT(g) so PE never parks mirror work in front
            # of the next group's PSUM fill.
            m3_done = [False]
            emit_z(0)
            emit_z(1)
            for gi in range(n_groups):
                emit_act(gi)
                if gi + 2 < n_groups:
                    emit_z(gi + 2)
                emit_tail(gi)
                if ones_i == n_ones and not m3_done[0]:
                    m3_done[0] = True
                    nc.vector.tensor_copy(M3S[:], M23[32:33, :])
                    nc.sync.dma_start(m3_out, M3S[:])
                if gi == n_groups // 2:
                    nc.sync.dma_start(accd_out[:, 0:gi], ACCD[:, 0:gi])

            assert mir_i == n_mir and ones_i == n_ones
            half = n_groups // 2
            nc.vector.tensor_copy(M23S[:], M23[0:2, :])
            nc.sync.dma_start(m23_out, M23S[:])
            nc.sync.dma_start(accd_out[:, half:n_groups - 1],
                              ACCD[:, half:n_groups - 1])
            nc.scalar.dma_start(accd_out[:, n_groups - 1:n_groups],
                                ACCD[:, n_groups - 1:n_groups])

    nc.compile()
    return nc, routes, n_groups


def _limbs_f8(x, n=NLIMB):
    x = np.asarray(x, np.float64)
    out = []
    r = x
    for _ in range(n):
        l = r.astype(F8)
        out.append(l)
        r = r - l.astype(np.float64)
    return out


def _limbs2(x):
    x = np.asarray(x, np.float64)
    l1 = x.astype(BF16)
    l2 = (x - l1.astype(np.float64)).astype(BF16)
    return l1, l2


def _build_rows(feat, s):
    """fp8 limb-pair rows for z = -0.5*d2 + log s_i + log1p(-s_j).
    Per-dim interleave keeps PE partial sums small for near pairs."""
    sq_d = feat * feat
    s64 = np.asarray(s, np.float64)
    with np.errstate(divide="ignore"):
        logs = np.maximum(np.log(s64), -500.0)
        lp = np.maximum(np.log1p(-s64), -500.0)
    a_rows, b_rows = [], []
    pairs = [(i, j) for i in range(1, NLIMB + 1) for j in range(1, NLIMB + 1)
             if i + j <= NLIMB + 1]
    ones = np.ones(N, np.float64)
    two = (ones * 2.0).astype(F8)
    for d in range(5):
        for l in _limbs_f8(-0.5 * sq_d[:, d] * 0.5):
            a_rows.append(two)
            b_rows.append(l)
        fl = _limbs_f8(feat[:, d])
        for (li, lj) in pairs:
            p = 2 * (lj - li)
            a_rows.append((fl[li - 1].astype(np.float64) * 2.0**-p).astype(F8))
            b_rows.append((fl[lj - 1].astype(np.float64) * 2.0**p).astype(F8))
        for l in _limbs_f8(-0.5 * sq_d[:, d] * 0.5):
            a_rows.append(l)
            b_rows.append(two)
    for l in _limbs_f8(logs * 0.5):
        a_rows.append(l)
        b_rows.append(two)
    for l in _limbs_f8(lp * 0.5):
        a_rows.append(two)
        b_rows.append(l)
    a = np.stack(a_rows)
    b = np.stack(b_rows)
    assert a.shape[0] == 2 * KP, a.shape
    return a, b


def _prep_inputs(input, image):
    s = np.asarray(input, np.float32).reshape(N).astype(np.float64)
    img = np.asarray(image, np.float32).reshape(3, N).astype(np.float64)
    yy, xx = np.meshgrid(
        np.arange(H, dtype=np.float64), np.arange(W, dtype=np.float64),
        indexing="ij")
    pos = np.stack([xx, yy], -1).reshape(N, 2) / SIGMA_XY
    feat = np.concatenate([pos, img.T / SIGMA_RGB], 1)

    a_all, b_all = _build_rows(feat, s)     # [2*KP, N] fp8

    r_full = np.minimum(s / np.maximum(1.0 - s, 1e-300), 1e30).astype(BF16)
    h_full = np.minimum((1.0 - s) / np.maximum(s, 1e-300), 1e30)

    in_maps = []
    for k in range(N_CORES):
        own = [(k + 8 * t) % NSLAB for t in range(T_SLABS)]
        rot = [(k + m) % NSLAB for m in range(NSLAB)]
        rows = np.concatenate(
            [np.arange(a0 * 128, (a0 + 1) * 128) for a0 in own])
        cols = np.concatenate(
            [np.arange(m0 * 128, (m0 + 1) * 128) for m0 in rot])
        h1, h2 = _limbs2(h_full[rows])
        h_packed = np.stack(
            [h1.reshape(T_SLABS, 128), h2.reshape(T_SLABS, 128)], 1)
        h_arr = np.ascontiguousarray(
            h_packed.reshape(T_SLABS * 2, 128).T.astype(BF16))
        a_k = a_all[:, rows].reshape(KP, 2, T_SLABS * 128)
        b_k = b_all[:, cols].reshape(KP, 2, N)
        in_maps.append(
            {
                "ab0_src": np.ascontiguousarray(
                    np.concatenate([a_k[:, :, 0:128], b_k[:, :, 128:1664]],
                                   axis=2)),
                "a_src": np.ascontiguousarray(
                    a_all[:, rows].reshape(KP, 2, T_SLABS * 128)),
                "b_src": np.ascontiguousarray(
                    b_all[:, cols].reshape(KP, 2, N)),
                "r_src": np.ascontiguousarray(r_full[cols])[None, :],
                "h_src": h_arr,
            }
        )
    return in_maps


def _get_module():
    if "nc" not in _cached:
        _cached["nc"], _cached["routes"], _cached["n_groups"] = \
            _build_module()
    return _cached["nc"], _cached["routes"], _cached["n_groups"]


def _run(in_maps, **kwargs):
    nc, _, _ = _get_module()
    return bass_utils.run_bass_kernel_spmd(
        nc, in_maps, core_ids=list(range(N_CORES)), **kwargs
    )


def kernel(input, image):
    assert input.shape == (1, 1, H, W) and image.shape == (1, 3, H, W)
    nc, routes, n_groups = _get_module()
    in_maps = _prep_inputs(input, image)
    res = _run(in_maps)
    act_cols = [gi for gi in range(n_groups) if routes[gi] == "act"]
    total = 0.0
    for k in range(N_CORES):
        r = res.results[k]
        total += r["accd_out"][:, act_cols].sum(dtype=np.float64)
        total += r["m23_out"].sum(dtype=np.float64)
        total += r["m3_out"].sum(dtype=np.float64)
    return np.array(total / N, dtype=np.float32)


# revision 42
# speedup vs baseline: 1.1987x; 1.0172x over previous
"""Dense-CRF relaxed Potts loss on 8 TRN2 NeuronCores — fp8 DoubleRow version.

v3: fp8e4m3 DoubleRow z-matmul (0.5 PE cycles/row), log s_i folded into the
matmul so activations need no per-slab bias and groups can span slabs
(fewer, larger ACT instructions), and direct row-sums load-balanced between
ACT accum_out and a PE ones-contraction into a second PSUM accumulator.

Math: loss*N = sum_ij s_i W_ij (1-s_j).  72 slabs of 128 rows; core k owns
slabs {k+8t}. Column data is rotated by k slabs so the SPMD program is
core-independent. Off-diagonal slab pairs are processed once (offsets
d=1..35 relative to the owning slab): the 'direct' term s_i W (1-s_j) comes
straight out of T = exp(z); the mirrored term (1-s_i) W s_j = h_i T_ij r_j
is computed as a DVE elementwise T*R followed by a PE contraction with the
2-limb h against a [2,512] PSUM accumulator (column-folded, partitions 0-1
of the 8th PSUM bank). Direct sums either ride the activation's accum_out
or a PE ones-contraction folded into partition 32 of the same bank. d=0 and
d=36 blocks are direct-only (d=36 appears once in each owning slab's run,
so both orientations are covered).

z = -0.5*d2 + log s_i + log1p(-s_j) is computed as a K=98 fp8e4m3
limb-pair decomposition (4 limbs/feature, cross pairs li+lj<=5, per-row
pow2 balancing so no limb under/overflows fp8; per-dim row ordering keeps
PE partial sums small for near pairs, where exp matters). fp32-grade z at
0.5 cycles/row via MatmulPerfMode.DoubleRow ([Kp=49, 2, .] layout).
"""

import numpy as np
import ml_dtypes

import concourse.bacc as bacc
import concourse.tile as tile
from concourse import mybir
import concourse.bass_utils as bass_utils

F8 = ml_dtypes.float8_e4m3fn
BF16 = ml_dtypes.bfloat16

SIGMA_XY = 15.0
SIGMA_RGB = 0.125
H = W = 96
N = H * W                   # 9216
N_CORES = 8
NSLAB = N // 128            # 72 slabs of 128 rows
T_SLABS = NSLAB // N_CORES  # 9 per core
D_MAX = 36
NLIMB = 4
KP = 49                     # fp8 row pairs: K=98
PSA = 16                    # psA pool blocks (4 banks)
PSB = 12                    # psB pool blocks (3 banks)
HEAD_SIZES = (4, 8)         # small first groups for a fast start

_cached = {}


def _stream():
    """Block stream: (t, m, mirror) with m the local column-slab index.
    Mirror blocks (d=1..35) slab-major first, then the 18 direct-only
    blocks (d=0 and d=36 of each slab) at the tail."""
    out = []
    for t in range(T_SLABS):
        for d in range(1, D_MAX):
            out.append((t, (8 * t + d) % NSLAB, True))
    for t in range(T_SLABS):
        out.append((t, 8 * t, False))
        out.append((t, (8 * t + D_MAX) % NSLAB, False))
    return out


def _plan():
    """Compile-time schedule.

    Returns list of group dicts:
      size, parity, zchunks [(o, nb, t, m0)], mpieces [(o, nb, t, m0)]
    where o/nb are block offsets/counts inside the group. zchunks respect
    PSUM bank alignment (cannot cross o%4 boundaries) and column/slab
    contiguity; mpieces are maximal same-slab column-contiguous mirror
    runs (SBUF-side, no bank constraint).
    """
    stream = _stream()
    groups = []
    i = 0
    parity = 0
    while i < len(stream):
        cap = (HEAD_SIZES[len(groups)] if len(groups) < len(HEAD_SIZES)
               else (PSA, PSB)[parity])
        nb = min(cap, len(stream) - i)
        blocks = stream[i:i + nb]

        def contiguous(x, y):
            return (x[0] == y[0] and x[2] == y[2]
                    and y[1] == x[1] + 1 and y[1] != 0)

        zchunks = []
        j = 0
        while j < nb:
            j2 = j + 1
            while (j2 < nb and j2 % 4 != 0
                   and contiguous(blocks[j2 - 1], blocks[j2])):
                j2 += 1
            zchunks.append((j, j2 - j, blocks[j][0], blocks[j][1]))
            j = j2
        mpieces = []
        j = 0
        while j < nb:
            if not blocks[j][2]:
                j += 1
                continue
            j2 = j + 1
            while (j2 < nb and blocks[j2][2]
                   and contiguous(blocks[j2 - 1], blocks[j2])):
                j2 += 1
            mpieces.append((j, j2 - j, blocks[j][0], blocks[j][1]))
            j = j2
        groups.append(dict(size=nb, parity=parity, zchunks=zchunks,
                           mpieces=mpieces))
        i += nb
        parity ^= 1
    return groups


def _routes(groups):
    """Direct-sum route per group: 'act' (accum_out) or 'pe' (ones
    contraction into M23 partition 32). PE takes mid-stream groups; ACT
    keeps the head (PE is cold) and the tail (short epilogue)."""
    n = len(groups)
    routes = []
    for gi in range(n):
        if gi < 2 or gi >= n - 3:
            routes.append("act")
        elif gi == 2 or 5 <= gi <= 15:
            routes.append("pool")
        else:
            routes.append("pe")
    return routes


def _build_module():
    groups = _plan()
    routes = _routes(groups)
    n_groups = len(groups)

    n_mir = sum(-(-p[1] * 128 // 512) for g in groups for p in g["mpieces"])
    n_ones = sum(-(-g["size"] * 128 // 512)
                 for gi, g in enumerate(groups) if routes[gi] == "pe")

    nc = bacc.Bacc(
        "TRN2",
        target_bir_lowering=False,
        debug=False,
        enable_asserts=False,
        num_devices=N_CORES,
    )
    f32 = mybir.dt.float32
    bf = mybir.dt.bfloat16
    f8 = mybir.dt.float8e4

    ab0_src = nc.dram_tensor("ab0_src", [KP, 2, 1664], f8,
                             kind="ExternalInput").ap()
    a_src = nc.dram_tensor("a_src", [KP, 2, T_SLABS * 128], f8,
                           kind="ExternalInput").ap()
    b_src = nc.dram_tensor("b_src", [KP, 2, N], f8, kind="ExternalInput").ap()
    r_src = nc.dram_tensor("r_src", [1, N], bf, kind="ExternalInput").ap()
    h_src = nc.dram_tensor("h_src", [128, 2 * T_SLABS], bf,
                           kind="ExternalInput").ap()
    accd_out = nc.dram_tensor("accd_out", [128, n_groups], f32,
                              kind="ExternalOutput").ap()
    m23_out = nc.dram_tensor("m23_out", [2, 512], f32,
                             kind="ExternalOutput").ap()
    m3_out = nc.dram_tensor("m3_out", [1, 512], f32,
                            kind="ExternalOutput").ap()
    dsum_out = nc.dram_tensor("dsum_out", [1, n_groups], f32,
                              kind="ExternalOutput").ap()

    with tile.TileContext(nc) as tc:
        with (
            tc.tile_pool(name="singles", bufs=1) as singles,
            tc.tile_pool(name="psA", bufs=1, space="PSUM") as psA_pool,
            tc.tile_pool(name="psB", bufs=1, space="PSUM") as psB_pool,
            tc.tile_pool(name="m23ps", bufs=1, space="PSUM") as m23_pool,
            tc.tile_pool(name="tpool", bufs=12) as t_pool,
            tc.tile_pool(name="trpool", bufs=4) as tr_pool,
        ):
            AB0 = singles.tile([KP, 2, 1664], f8)
            A = singles.tile([KP, 2, T_SLABS * 128], f8)
            B = singles.tile([KP, 2, N], f8)
            R = singles.tile([128, N], bf)
            Hh = singles.tile([128, 2 * T_SLABS], bf)
            ONES = singles.tile([128, 1], bf)
            ACCD = singles.tile([128, n_groups], f32)
            M23 = m23_pool.tile([33, 512], f32)
            M23S = singles.tile([2, 512], f32)
            M3S = singles.tile([1, 512], f32)
            DSUMP = singles.tile([1, n_groups], f32)

            # trigger the ACT exp table load immediately
            DUM = singles.tile([128, 1], f32)
            nc.gpsimd.memset(DUM[:], 0.0)
            nc.gpsimd.memset(ONES[:], 1.0)
            nc.gpsimd.memset(ACCD[:], 0.0)
            nc.gpsimd.memset(DSUMP[:], 0.0)
            nc.scalar.activation(
                DUM[:], DUM[:], mybir.ActivationFunctionType.Exp,
                bias=0.0, scale=0.0,
            )
            # staged DMAs: B chunks stream on the SP queue in consumption
            # order; A/R/Hh go via the Pool (SWDGE) queue in parallel so
            # the head isn't serialized on one sequencer.
            nc.sync.dma_start(AB0[:], ab0_src)
            nc.sync.dma_start(B[:, :, 128:1664], b_src[:, :, 128:1664])
            nc.sync.dma_start(B[:, :, 1664:3712], b_src[:, :, 1664:3712])
            nc.sync.dma_start(Hh[:], h_src)
            nc.sync.dma_start(B[:, :, 3712:4608], b_src[:, :, 3712:4608])
            nc.sync.dma_start(B[:, :, 4608:6656], b_src[:, :, 4608:6656])
            nc.sync.dma_start(B[:, :, 6656:N], b_src[:, :, 6656:N])
            nc.sync.dma_start(B[:, :, 0:128], b_src[:, :, 0:128])
            nc.gpsimd.dma_start(A[:], a_src)
            nc.gpsimd.dma_start(R[:, 128:1664],
                                r_src[:, 128:1664].broadcast_to((128, 1536)))
            nc.gpsimd.dma_start(R[:, 1664:4608],
                                r_src[:, 1664:4608].broadcast_to((128, 2944)))
            nc.gpsimd.dma_start(R[:, 4608:6912],
                                r_src[:, 4608:6912].broadcast_to((128, 2304)))
            nc.gpsimd.dma_start(R[:, 6912:N],
                                r_src[:, 6912:N].broadcast_to((128, 2304)))
            nc.gpsimd.dma_start(R[:, 0:128],
                                r_src[:, 0:128].broadcast_to((128, 128)))

            mir_i = 0
            ones_i = 0
            pts = {}
            t_tiles = {}

            def emit_z(gi):
                g = groups[gi]
                parity = g["parity"]
                pool_g = psA_pool if parity == 0 else psB_pool
                cap = (PSA, PSB)[parity]
                pt = pool_g.tile([128, cap * 128], f32, tag=f"ps{parity}")
                pts[gi] = pt
                for (o, nb, t, m0) in g["zchunks"]:
                    q0, qw, c0 = o * 128, nb * 128, m0 * 128
                    if gi <= 1:
                        lhsT, rhs = AB0[:, :, 0:128], AB0[:, :, c0:c0 + qw]
                    else:
                        lhsT = A[:, :, t * 128:(t + 1) * 128]
                        rhs = B[:, :, c0:c0 + qw]
                    nc.tensor.matmul(
                        pt[:, q0:q0 + qw],
                        lhsT=lhsT,
                        rhs=rhs,
                        start=True, stop=True,
                        perf_mode=mybir.MatmulPerfMode.DoubleRow,
                    )

            def emit_act(gi):
                g = groups[gi]
                width = g["size"] * 128
                T = t_pool.tile([128, PSA * 128], bf, tag="T")
                t_tiles[gi] = T
                accum = ACCD[:, gi:gi + 1] if routes[gi] == "act" else None
                nc.scalar.activation(
                    T[:, 0:width], pts.pop(gi)[:, 0:width],
                    mybir.ActivationFunctionType.Exp,
                    bias=0.0, scale=1.0,
                    accum_out=accum,
                )

            def emit_tail(gi):
                nonlocal mir_i, ones_i
                g = groups[gi]
                width = g["size"] * 128
                T = t_tiles.pop(gi)
                for (o, nb, t, m0) in g["mpieces"]:
                    o0, w, c0 = o * 128, nb * 128, m0 * 128
                    TR = tr_pool.tile([128, PSA * 128], bf, tag="TR")
                    nc.vector.tensor_tensor(
                        TR[:, o0:o0 + w], T[:, o0:o0 + w],
                        R[:, c0:c0 + w], mybir.AluOpType.mult,
                    )
                    for q in range(0, w, 512):
                        qw = min(512, w - q)
                        nc.tensor.matmul(
                            M23[0:2, 0:qw],
                            lhsT=Hh[:, 2 * t:2 * t + 2],
                            rhs=TR[:, o0 + q:o0 + q + qw],
                            start=(mir_i == 0),
                            stop=(mir_i == n_mir - 1),
                            skip_group_check=True,
                        )
                        mir_i += 1
                if routes[gi] == "pe":
                    for q in range(0, width, 512):
                        qw = min(512, width - q)
                        nc.tensor.matmul(
                            M23[32:33, 0:qw],
                            lhsT=ONES[:],
                            rhs=T[:, q:q + qw],
                            start=(ones_i == 0),
                            stop=(ones_i == n_ones - 1),
                            skip_group_check=True,
                            tile_position=(0, 32),
                        )
                        ones_i += 1
                elif routes[gi] == "dve":
                    nc.vector.tensor_reduce(
                        ACCD[:, gi:gi + 1], T[:, 0:width],
                        mybir.AxisListType.X, mybir.AluOpType.add,
                    )
                elif routes[gi] == "pool":
                    nc.gpsimd.tensor_reduce(
                        DSUMP[0:1, gi:gi + 1], T[:, 0:width],
                        mybir.AxisListType.XYZWC, mybir.AluOpType.add,
                    )

            # software-pipelined emission: z(g+2) goes to the PE queue
            # right after ACT(g) so PE never parks mirror work in front
            # of the next group's PSUM fill.
            m3_done = [False]
            emit_z(0)
            emit_z(1)
            for gi in range(n_groups):
                emit_act(gi)
                if gi + 2 < n_groups:
                    emit_z(gi + 2)
                emit_tail(gi)
                if ones_i == n_ones and not m3_done[0]:
                    m3_done[0] = True
                    nc.vector.tensor_copy(M3S[:], M23[32:33, :])
                    nc.sync.dma_start(m3_out, M3S[:])
                if gi == n_groups // 2:
                    nc.sync.dma_start(accd_out[:, 0:gi], ACCD[:, 0:gi])

            assert mir_i == n_mir and ones_i == n_ones
            half = n_groups // 2
            nc.vector.tensor_copy(M23S[:], M23[0:2, :])
            nc.sync.dma_start(dsum_out, DSUMP[:])
            nc.sync.dma_start(m23_out, M23S[:])
            nc.sync.dma_start(accd_out[:, half:n_groups - 1],
                              ACCD[:, half:n_groups - 1])
            nc.scalar.dma_start(accd_out[:, n_groups - 1:n_groups],
                                ACCD[:, n_groups - 1:n_groups])

    nc.compile()
    return nc, routes, n_groups


def _limbs_f8(x, n=NLIMB):
    x = np.asarray(x, np.float64)
    out = []
    r = x
    for _ in range(n):
        l = r.astype(F8)
        out.append(l)
        r = r - l.astype(np.float64)
    return out


def _limbs2(x):
    x = np.asarray(x, np.float64)
    l1 = x.astype(BF16)
    l2 = (x - l1.astype(np.float64)).astype(BF16)
    return l1, l2


def _build_rows(feat, s):
    """fp8 limb-pair rows for z = -0.5*d2 + log s_i + log1p(-s_j).
    Per-dim interleave keeps PE partial sums small for near pairs."""
    sq_d = feat * feat
    s64 = np.asarray(s, np.float64)
    with np.errstate(divide="ignore"):
        logs = np.maximum(np.log(s64), -500.0)
        lp = np.maximum(np.log1p(-s64), -500.0)
    a_rows, b_rows = [], []
    pairs = [(i, j) for i in range(1, NLIMB + 1) for j in range(1, NLIMB + 1)
             if i + j <= NLIMB + 1]
    ones = np.ones(N, np.float64)
    two = (ones * 2.0).astype(F8)
    for d in range(5):
        for l in _limbs_f8(-0.5 * sq_d[:, d] * 0.5):
            a_rows.append(two)
            b_rows.append(l)
        fl = _limbs_f8(feat[:, d])
        for (li, lj) in pairs:
            p = 2 * (lj - li)
            a_rows.append((fl[li - 1].astype(np.float64) * 2.0**-p).astype(F8))
            b_rows.append((fl[lj - 1].astype(np.float64) * 2.0**p).astype(F8))
        for l in _limbs_f8(-0.5 * sq_d[:, d] * 0.5):
            a_rows.append(l)
            b_rows.append(two)
    for l in _limbs_f8(logs * 0.5):
        a_rows.append(l)
        b_rows.append(two)
    for l in _limbs_f8(lp * 0.5):
        a_rows.append(two)
        b_rows.append(l)
    a = np.stack(a_rows)
    b = np.stack(b_rows)
    assert a.shape[0] == 2 * KP, a.shape
    return a, b


def _prep_inputs(input, image):
    s = np.asarray(input, np.float32).reshape(N).astype(np.float64)
    img = np.asarray(image, np.float32).reshape(3, N).astype(np.float64)
    yy, xx = np.meshgrid(
        np.arange(H, dtype=np.float64), np.arange(W, dtype=np.float64),
        indexing="ij")
    pos = np.stack([xx, yy], -1).reshape(N, 2) / SIGMA_XY
    feat = np.concatenate([pos, img.T / SIGMA_RGB], 1)

    a_all, b_all = _build_rows(feat, s)     # [2*KP, N] fp8

    r_full = np.minimum(s / np.maximum(1.0 - s, 1e-300), 1e30).astype(BF16)
    h_full = np.minimum((1.0 - s) / np.maximum(s, 1e-300), 1e30)

    in_maps = []
    for k in range(N_CORES):
        own = [(k + 8 * t) % NSLAB for t in range(T_SLABS)]
        rot = [(k + m) % NSLAB for m in range(NSLAB)]
        rows = np.concatenate(
            [np.arange(a0 * 128, (a0 + 1) * 128) for a0 in own])
        cols = np.concatenate(
            [np.arange(m0 * 128, (m0 + 1) * 128) for m0 in rot])
        h1, h2 = _limbs2(h_full[rows])
        h_packed = np.stack(
            [h1.reshape(T_SLABS, 128), h2.reshape(T_SLABS, 128)], 1)
        h_arr = np.ascontiguousarray(
            h_packed.reshape(T_SLABS * 2, 128).T.astype(BF16))
        a_k = a_all[:, rows].reshape(KP, 2, T_SLABS * 128)
        b_k = b_all[:, cols].reshape(KP, 2, N)
        in_maps.append(
            {
                "ab0_src": np.ascontiguousarray(
                    np.concatenate([a_k[:, :, 0:128], b_k[:, :, 128:1664]],
                                   axis=2)),
                "a_src": np.ascontiguousarray(
                    a_all[:, rows].reshape(KP, 2, T_SLABS * 128)),
                "b_src": np.ascontiguousarray(
                    b_all[:, cols].reshape(KP, 2, N)),
                "r_src": np.ascontiguousarray(r_full[cols])[None, :],
                "h_src": h_arr,
            }
        )
    return in_maps


def _get_module():
    if "nc" not in _cached:
        _cached["nc"], _cached["routes"], _cached["n_groups"] = \
            _build_module()
    return _cached["nc"], _cached["routes"], _cached["n_groups"]


def _run(in_maps, **kwargs):
    nc, _, _ = _get_module()
    return bass_utils.run_bass_kernel_spmd(
        nc, in_maps, core_ids=list(range(N_CORES)), **kwargs
    )


def kernel(input, image):
    assert input.shape == (1, 1, H, W) and image.shape == (1, 3, H, W)
    nc, routes, n_groups = _get_module()
    in_maps = _prep_inputs(input, image)
    res = _run(in_maps)
    act_cols = [gi for gi in range(n_groups) if routes[gi] in ("act", "dve")]
    pool_cols = [gi for gi in range(n_groups) if routes[gi] == "pool"]
    total = 0.0
    for k in range(N_CORES):
        r = res.results[k]
        total += r["accd_out"][:, act_cols].sum(dtype=np.float64)
        total += r["dsum_out"][0, pool_cols].sum(dtype=np.float64)
        total += r["m23_out"].sum(dtype=np.float64)
        total += r["m3_out"].sum(dtype=np.float64)
    return np.array(total / N, dtype=np.float32)


# revision 45
# speedup vs baseline: 1.2054x; 1.0056x over previous
"""Dense-CRF relaxed Potts loss on 8 TRN2 NeuronCores — fp8 DoubleRow version.

v3: fp8e4m3 DoubleRow z-matmul (0.5 PE cycles/row), log s_i folded into the
matmul so activations need no per-slab bias and groups can span slabs
(fewer, larger ACT instructions), and direct row-sums load-balanced between
ACT accum_out and a PE ones-contraction into a second PSUM accumulator.

Math: loss*N = sum_ij s_i W_ij (1-s_j).  72 slabs of 128 rows; core k owns
slabs {k+8t}. Column data is rotated by k slabs so the SPMD program is
core-independent. Off-diagonal slab pairs are processed once (offsets
d=1..35 relative to the owning slab): the 'direct' term s_i W (1-s_j) comes
straight out of T = exp(z); the mirrored term (1-s_i) W s_j = h_i T_ij r_j
is computed as a DVE elementwise T*R followed by a PE contraction with the
2-limb h against a [2,512] PSUM accumulator (column-folded, partitions 0-1
of the 8th PSUM bank). Direct sums either ride the activation's accum_out
or a PE ones-contraction folded into partition 32 of the same bank. d=0 and
d=36 blocks are direct-only (d=36 appears once in each owning slab's run,
so both orientations are covered).

z = -0.5*d2 + log s_i + log1p(-s_j) is computed as a K=98 fp8e4m3
limb-pair decomposition (4 limbs/feature, cross pairs li+lj<=5, per-row
pow2 balancing so no limb under/overflows fp8; per-dim row ordering keeps
PE partial sums small for near pairs, where exp matters). fp32-grade z at
0.5 cycles/row via MatmulPerfMode.DoubleRow ([Kp=49, 2, .] layout).
"""

import numpy as np
import ml_dtypes

import concourse.bacc as bacc
import concourse.tile as tile
from concourse import mybir
import concourse.bass_utils as bass_utils

F8 = ml_dtypes.float8_e4m3fn
BF16 = ml_dtypes.bfloat16

SIGMA_XY = 15.0
SIGMA_RGB = 0.125
H = W = 96
N = H * W                   # 9216
N_CORES = 8
NSLAB = N // 128            # 72 slabs of 128 rows
T_SLABS = NSLAB // N_CORES  # 9 per core
D_MAX = 36
NLIMB = 4
KP = 49                     # fp8 row pairs: K=98
PSA = 16                    # psA pool blocks (4 banks)
PSB = 12                    # psB pool blocks (3 banks)
HEAD_SIZES = (4, 8)         # small first groups for a fast start

_cached = {}


def _stream():
    """Block stream: (t, m, mirror) with m the local column-slab index.
    Mirror blocks (d=1..35) slab-major first, then the 18 direct-only
    blocks (d=0 and d=36 of each slab) at the tail."""
    out = []
    for t in range(T_SLABS):
        for d in range(1, D_MAX):
            out.append((t, (8 * t + d) % NSLAB, True))
    for t in range(T_SLABS):
        out.append((t, 8 * t, False))
        out.append((t, (8 * t + D_MAX) % NSLAB, False))
    return out


def _plan():
    """Compile-time schedule.

    Returns list of group dicts:
      size, parity, zchunks [(o, nb, t, m0)], mpieces [(o, nb, t, m0)]
    where o/nb are block offsets/counts inside the group. zchunks respect
    PSUM bank alignment (cannot cross o%4 boundaries) and column/slab
    contiguity; mpieces are maximal same-slab column-contiguous mirror
    runs (SBUF-side, no bank constraint).
    """
    stream = _stream()
    groups = []
    i = 0
    parity = 0
    while i < len(stream):
        cap = (HEAD_SIZES[len(groups)] if len(groups) < len(HEAD_SIZES)
               else (PSA, PSB)[parity])
        nb = min(cap, len(stream) - i)
        blocks = stream[i:i + nb]

        def contiguous(x, y):
            return (x[0] == y[0] and x[2] == y[2]
                    and y[1] == x[1] + 1 and y[1] != 0)

        zchunks = []
        j = 0
        while j < nb:
            j2 = j + 1
            while (j2 < nb and j2 % 4 != 0
                   and contiguous(blocks[j2 - 1], blocks[j2])):
                j2 += 1
            zchunks.append((j, j2 - j, blocks[j][0], blocks[j][1]))
            j = j2
        mpieces = []
        j = 0
        while j < nb:
            if not blocks[j][2]:
                j += 1
                continue
            j2 = j + 1
            while (j2 < nb and blocks[j2][2]
                   and contiguous(blocks[j2 - 1], blocks[j2])):
                j2 += 1
            mpieces.append((j, j2 - j, blocks[j][0], blocks[j][1]))
            j = j2
        groups.append(dict(size=nb, parity=parity, zchunks=zchunks,
                           mpieces=mpieces))
        i += nb
        parity ^= 1
    return groups


def _routes(groups):
    """Direct-sum route per group: 'act' (accum_out) or 'pe' (ones
    contraction into M23 partition 32). PE takes mid-stream groups; ACT
    keeps the head (PE is cold) and the tail (short epilogue)."""
    n = len(groups)
    routes = []
    for gi in range(n):
        if gi < 2 or gi >= n - 3:
            routes.append("act")
        elif gi == 2 or 5 <= gi <= 16:
            routes.append("pool")
        else:
            routes.append("pe")
    return routes


def _build_module():
    groups = _plan()
    routes = _routes(groups)
    n_groups = len(groups)

    n_mir = sum(-(-p[1] * 128 // 512) for g in groups for p in g["mpieces"])
    n_ones = sum(-(-g["size"] * 128 // 512)
                 for gi, g in enumerate(groups) if routes[gi] == "pe")

    nc = bacc.Bacc(
        "TRN2",
        target_bir_lowering=False,
        debug=False,
        enable_asserts=False,
        num_devices=N_CORES,
    )
    f32 = mybir.dt.float32
    bf = mybir.dt.bfloat16
    f8 = mybir.dt.float8e4

    ab0_src = nc.dram_tensor("ab0_src", [KP, 2, 1664], f8,
                             kind="ExternalInput").ap()
    a_src = nc.dram_tensor("a_src", [KP, 2, T_SLABS * 128], f8,
                           kind="ExternalInput").ap()
    b_src = nc.dram_tensor("b_src", [KP, 2, N], f8, kind="ExternalInput").ap()
    r_src = nc.dram_tensor("r_src", [1, N], bf, kind="ExternalInput").ap()
    h_src = nc.dram_tensor("h_src", [128, 2 * T_SLABS], bf,
                           kind="ExternalInput").ap()
    accd_out = nc.dram_tensor("accd_out", [128, n_groups], f32,
                              kind="ExternalOutput").ap()
    m23_out = nc.dram_tensor("m23_out", [2, 512], f32,
                             kind="ExternalOutput").ap()
    m3_out = nc.dram_tensor("m3_out", [1, 512], f32,
                            kind="ExternalOutput").ap()
    dsum_out = nc.dram_tensor("dsum_out", [1, n_groups], f32,
                              kind="ExternalOutput").ap()

    with tile.TileContext(nc) as tc:
        with (
            tc.tile_pool(name="singles", bufs=1) as singles,
            tc.tile_pool(name="psA", bufs=1, space="PSUM") as psA_pool,
            tc.tile_pool(name="psB", bufs=1, space="PSUM") as psB_pool,
            tc.tile_pool(name="m23ps", bufs=1, space="PSUM") as m23_pool,
            tc.tile_pool(name="tpool", bufs=12) as t_pool,
            tc.tile_pool(name="trpool", bufs=4) as tr_pool,
        ):
            AB0 = singles.tile([KP, 2, 1664], f8)
            A = singles.tile([KP, 2, T_SLABS * 128], f8)
            B = singles.tile([KP, 2, N], f8)
            R = singles.tile([128, N], bf)
            Hh = singles.tile([128, 2 * T_SLABS], bf)
            ONES = singles.tile([128, 1], bf)
            ACCD = singles.tile([128, n_groups], f32)
            M23 = m23_pool.tile([33, 512], f32)
            M23S = singles.tile([2, 512], f32)
            M3S = singles.tile([1, 512], f32)
            DSUMP = singles.tile([1, n_groups], f32)

            # trigger the ACT exp table load immediately
            DUM = singles.tile([128, 1], f32)
            nc.gpsimd.memset(DUM[:], 0.0)
            nc.gpsimd.memset(ONES[:], 1.0)
            nc.gpsimd.memset(ACCD[:], 0.0)
            nc.gpsimd.memset(DSUMP[:], 0.0)
            nc.scalar.activation(
                DUM[:], DUM[:], mybir.ActivationFunctionType.Exp,
                bias=0.0, scale=0.0,
            )
            # staged DMAs: B chunks stream on the SP queue in consumption
            # order; A/R/Hh go via the Pool (SWDGE) queue in parallel so
            # the head isn't serialized on one sequencer.
            nc.sync.dma_start(AB0[:], ab0_src)
            nc.sync.dma_start(B[:, :, 128:1664], b_src[:, :, 128:1664])
            nc.sync.dma_start(B[:, :, 1664:3712], b_src[:, :, 1664:3712])
            nc.sync.dma_start(Hh[:], h_src)
            nc.sync.dma_start(B[:, :, 3712:4608], b_src[:, :, 3712:4608])
            nc.sync.dma_start(B[:, :, 4608:6656], b_src[:, :, 4608:6656])
            nc.sync.dma_start(B[:, :, 6656:N], b_src[:, :, 6656:N])
            nc.sync.dma_start(B[:, :, 0:128], b_src[:, :, 0:128])
            nc.gpsimd.dma_start(A[:], a_src)
            nc.gpsimd.dma_start(R[:, 128:1664],
                                r_src[:, 128:1664].broadcast_to((128, 1536)))
            nc.gpsimd.dma_start(R[:, 1664:4608],
                                r_src[:, 1664:4608].broadcast_to((128, 2944)))
            nc.gpsimd.dma_start(R[:, 4608:6912],
                                r_src[:, 4608:6912].broadcast_to((128, 2304)))
            nc.gpsimd.dma_start(R[:, 6912:N],
                                r_src[:, 6912:N].broadcast_to((128, 2304)))
            nc.gpsimd.dma_start(R[:, 0:128],
                                r_src[:, 0:128].broadcast_to((128, 128)))

            mir_i = 0
            ones_i = 0
            pts = {}
            t_tiles = {}

            def emit_z(gi):
                g = groups[gi]
                parity = g["parity"]
                pool_g = psA_pool if parity == 0 else psB_pool
                cap = (PSA, PSB)[parity]
                pt = pool_g.tile([128, cap * 128], f32, tag=f"ps{parity}")
                pts[gi] = pt
                for (o, nb, t, m0) in g["zchunks"]:
                    q0, qw, c0 = o * 128, nb * 128, m0 * 128
                    if gi <= 1:
                        lhsT, rhs = AB0[:, :, 0:128], AB0[:, :, c0:c0 + qw]
                    else:
                        lhsT = A[:, :, t * 128:(t + 1) * 128]
                        rhs = B[:, :, c0:c0 + qw]
                    nc.tensor.matmul(
                        pt[:, q0:q0 + qw],
                        lhsT=lhsT,
                        rhs=rhs,
                        start=True, stop=True,
                        perf_mode=mybir.MatmulPerfMode.DoubleRow,
                    )

            def emit_act(gi):
                g = groups[gi]
                width = g["size"] * 128
                T = t_pool.tile([128, PSA * 128], bf, tag="T")
                t_tiles[gi] = T
                accum = ACCD[:, gi:gi + 1] if routes[gi] == "act" else None
                nc.scalar.activation(
                    T[:, 0:width], pts.pop(gi)[:, 0:width],
                    mybir.ActivationFunctionType.Exp,
                    bias=0.0, scale=1.0,
                    accum_out=accum,
                )

            def emit_tail(gi):
                nonlocal mir_i, ones_i
                g = groups[gi]
                width = g["size"] * 128
                T = t_tiles.pop(gi)
                for (o, nb, t, m0) in g["mpieces"]:
                    o0, w, c0 = o * 128, nb * 128, m0 * 128
                    TR = tr_pool.tile([128, PSA * 128], bf, tag="TR")
                    nc.vector.tensor_tensor(
                        TR[:, o0:o0 + w], T[:, o0:o0 + w],
                        R[:, c0:c0 + w], mybir.AluOpType.mult,
                    )
                    for q in range(0, w, 512):
                        qw = min(512, w - q)
                        nc.tensor.matmul(
                            M23[0:2, 0:qw],
                            lhsT=Hh[:, 2 * t:2 * t + 2],
                            rhs=TR[:, o0 + q:o0 + q + qw],
                            start=(mir_i == 0),
                            stop=(mir_i == n_mir - 1),
                            skip_group_check=True,
                        )
                        mir_i += 1
                if routes[gi] == "pe":
                    for q in range(0, width, 512):
                        qw = min(512, width - q)
                        nc.tensor.matmul(
                            M23[32:33, 0:qw],
                            lhsT=ONES[:],
                            rhs=T[:, q:q + qw],
                            start=(ones_i == 0),
                            stop=(ones_i == n_ones - 1),
                            skip_group_check=True,
                            tile_position=(0, 32),
                        )
                        ones_i += 1
                elif routes[gi] == "dve":
                    nc.vector.tensor_reduce(
                        ACCD[:, gi:gi + 1], T[:, 0:width],
                        mybir.AxisListType.X, mybir.AluOpType.add,
                    )
                elif routes[gi] == "pool":
                    nc.gpsimd.tensor_reduce(
                        DSUMP[0:1, gi:gi + 1], T[:, 0:width],
                        mybir.AxisListType.XYZWC, mybir.AluOpType.add,
                    )

            # software-pipelined emission: z(g+2) goes to the PE queue
            # right after ACT(g) so PE never parks mirror work in front
            # of the next group's PSUM fill.
            m3_done = [False]
            emit_z(0)
            emit_z(1)
            for gi in range(n_groups):
                emit_act(gi)
                if gi + 2 < n_groups:
                    emit_z(gi + 2)
                emit_tail(gi)
                if ones_i == n_ones and not m3_done[0]:
                    m3_done[0] = True
                    nc.vector.tensor_copy(M3S[:], M23[32:33, :])
                    nc.sync.dma_start(m3_out, M3S[:])
                if gi == n_groups // 2:
                    nc.sync.dma_start(accd_out[:, 0:gi], ACCD[:, 0:gi])

            assert mir_i == n_mir and ones_i == n_ones
            half = n_groups // 2
            nc.vector.tensor_copy(M23S[:], M23[0:2, :])
            nc.sync.dma_start(dsum_out, DSUMP[:])
            nc.sync.dma_start(m23_out, M23S[:])
            nc.sync.dma_start(accd_out[:, half:n_groups - 1],
                              ACCD[:, half:n_groups - 1])
            nc.scalar.dma_start(accd_out[:, n_groups - 1:n_groups],
                                ACCD[:, n_groups - 1:n_groups])

    nc.compile()
    return nc, routes, n_groups


def _limbs_f8(x, n=NLIMB):
    x = np.asarray(x, np.float64)
    out = []
    r = x
    for _ in range(n):
        l = r.astype(F8)
        out.append(l)
        r = r - l.astype(np.float64)
    return out


def _limbs2(x):
    x = np.asarray(x, np.float64)
    l1 = x.astype(BF16)
    l2 = (x - l1.astype(np.float64)).astype(BF16)
    return l1, l2


def _build_rows(feat, s):
    """fp8 limb-pair rows for z = -0.5*d2 + log s_i + log1p(-s_j).
    Per-dim interleave keeps PE partial sums small for near pairs."""
    sq_d = feat * feat
    s64 = np.asarray(s, np.float64)
    with np.errstate(divide="ignore"):
        logs = np.maximum(np.log(s64), -500.0)
        lp = np.maximum(np.log1p(-s64), -500.0)
    a_rows, b_rows = [], []
    pairs = [(i, j) for i in range(1, NLIMB + 1) for j in range(1, NLIMB + 1)
             if i + j <= NLIMB + 1]
    ones = np.ones(N, np.float64)
    two = (ones * 2.0).astype(F8)
    for d in range(5):
        for l in _limbs_f8(-0.5 * sq_d[:, d] * 0.5):
            a_rows.append(two)
            b_rows.append(l)
        fl = _limbs_f8(feat[:, d])
        for (li, lj) in pairs:
            p = 2 * (lj - li)
            a_rows.append((fl[li - 1].astype(np.float64) * 2.0**-p).astype(F8))
            b_rows.append((fl[lj - 1].astype(np.float64) * 2.0**p).astype(F8))
        for l in _limbs_f8(-0.5 * sq_d[:, d] * 0.5):
            a_rows.append(l)
            b_rows.append(two)
    for l in _limbs_f8(logs * 0.5):
        a_rows.append(l)
        b_rows.append(two)
    for l in _limbs_f8(lp * 0.5):
        a_rows.append(two)
        b_rows.append(l)
    a = np.stack(a_rows)
    b = np.stack(b_rows)
    assert a.shape[0] == 2 * KP, a.shape
    return a, b


def _prep_inputs(input, image):
    s = np.asarray(input, np.float32).reshape(N).astype(np.float64)
    img = np.asarray(image, np.float32).reshape(3, N).astype(np.float64)
    yy, xx = np.meshgrid(
        np.arange(H, dtype=np.float64), np.arange(W, dtype=np.float64),
        indexing="ij")
    pos = np.stack([xx, yy], -1).reshape(N, 2) / SIGMA_XY
    feat = np.concatenate([pos, img.T / SIGMA_RGB], 1)

    a_all, b_all = _build_rows(feat, s)     # [2*KP, N] fp8

    r_full = np.minimum(s / np.maximum(1.0 - s, 1e-300), 1e30).astype(BF16)
    h_full = np.minimum((1.0 - s) / np.maximum(s, 1e-300), 1e30)

    in_maps = []
    for k in range(N_CORES):
        own = [(k + 8 * t) % NSLAB for t in range(T_SLABS)]
        rot = [(k + m) % NSLAB for m in range(NSLAB)]
        rows = np.concatenate(
            [np.arange(a0 * 128, (a0 + 1) * 128) for a0 in own])
        cols = np.concatenate(
            [np.arange(m0 * 128, (m0 + 1) * 128) for m0 in rot])
        h1, h2 = _limbs2(h_full[rows])
        h_packed = np.stack(
            [h1.reshape(T_SLABS, 128), h2.reshape(T_SLABS, 128)], 1)
        h_arr = np.ascontiguousarray(
            h_packed.reshape(T_SLABS * 2, 128).T.astype(BF16))
        a_k = a_all[:, rows].reshape(KP, 2, T_SLABS * 128)
        b_k = b_all[:, cols].reshape(KP, 2, N)
        in_maps.append(
            {
                "ab0_src": np.ascontiguousarray(
                    np.concatenate([a_k[:, :, 0:128], b_k[:, :, 128:1664]],
                                   axis=2)),
                "a_src": np.ascontiguousarray(
                    a_all[:, rows].reshape(KP, 2, T_SLABS * 128)),
                "b_src": np.ascontiguousarray(
                    b_all[:, cols].reshape(KP, 2, N)),
                "r_src": np.ascontiguousarray(r_full[cols])[None, :],
                "h_src": h_arr,
            }
        )
    return in_maps


def _get_module():
    if "nc" not in _cached:
        _cached["nc"], _cached["routes"], _cached["n_groups"] = \
            _build_module()
    return _cached["nc"], _cached["routes"], _cached["n_groups"]


def _run(in_maps, **kwargs):
    nc, _, _ = _get_module()
    return bass_utils.run_bass_kernel_spmd(
        nc, in_maps, core_ids=list(range(N_CORES)), **kwargs
    )


def kernel(input, image):
    assert input.shape == (1, 1, H, W) and image.shape == (1, 3, H, W)
    nc, routes, n_groups = _get_module()
    in_maps = _prep_inputs(input, image)
    res = _run(in_maps)
    act_cols = [gi for gi in range(n_groups) if routes[gi] in ("act", "dve")]
    pool_cols = [gi for gi in range(n_groups) if routes[gi] == "pool"]
    total = 0.0
    for k in range(N_CORES):
        r = res.results[k]
        total += r["accd_out"][:, act_cols].sum(dtype=np.float64)
        total += r["dsum_out"][0, pool_cols].sum(dtype=np.float64)
        total += r["m23_out"].sum(dtype=np.float64)
        total += r["m3_out"].sum(dtype=np.float64)
    return np.array(total / N, dtype=np.float32)
